# revision 13
# baseline (speedup 1.0000x reference)
"""Causal self-attention (B=2, N=2048, E=1024, H=16, HD=64) on 8 trn2 NeuronCores.

Sharding: (batch, head-group) — core c handles batch c//4 and heads
4*(c%4) .. 4*(c%4)+3.  Each core computes its heads' QKV projections,
causal attention, and a partial out-projection over its 256 feature rows
of Wout; the host sums the 4 partials per batch and adds all biases that
are affine in the output (bout and the v-bias term, which is constant
because softmax rows sum to 1).

On-device layout avoids every transpose:
  - host feeds xT (E-major) so QK projections produce qT/kT [d, n] directly
  - S^T tiles [k, q] = kT-slice.T @ qT-slice (contraction over d)
  - exp on ScalarE (no max subtraction: logits are O(1) by construction)
  - PV uses expST tiles as lhsT with a ones-column appended to v to get the
    softmax denominator for free; normalization via gpsimd partition
    broadcast of the DVE reciprocal.
All matmuls run in float32r (full-rate fp32 mode, ~tf32 precision).

Schedule: attention is ScalarE(exp)-bound, so the v projection and the
second head-pair's qk projection are interleaved into head 0's attention
stream as PE filler; each head runs two query passes (q in [0,1024) then
[1024,2048)) so only two PV accumulators are live at a time (PSUM budget).
"""

import numpy as np

import concourse.bass as bass
import concourse.tile as tile
from concourse import bacc, mybir
from concourse import bass_utils

B, N, E, H = 2, 2048, 1024, 16
HD = 64
NCORES = 8
NE = E // 128      # 8 e-chunks
NK = N // 128      # 16 k-tiles / n-chunks
NQ = N // 512      # 4 q-chunks of 512
F32 = mybir.dt.float32
F32R = mybir.dt.float32r

_CACHE = {}


def _build_body(nc, tc, pools, dram, rep):
    xt_d, wq_d, wk_d, wv_d, wout_d, bqk_d, mask_d, ones_d, out_d = dram
    (pconst, pqk, pvext, psa, pesb, psmall, pout, ppv, pgen, pproj) = pools
    Exp = mybir.ActivationFunctionType.Exp

    # ---- constant loads (small first, then xt in 512-wide n-chunks) -----
    wq_sb, wk_sb, wv_sb = [], [], []
    for e in range(NE):
        for lst, src, nm in ((wq_sb, wq_d, "wq"), (wk_sb, wk_d, "wk"),
                             (wv_sb, wv_d, "wv")):
            t = pconst.tile([128, 256], F32R, tag=f"{nm}{e}", name=f"{nm}{e}")
            nc.sync.dma_start(t[:], src[e])
            lst.append(t)
    bias_sb = {}
    for p in range(2):
        for i, nm in enumerate(("bq", "bk")):
            t = pconst.tile([128, 1], F32, tag=f"{nm}{p}", name=f"{nm}{p}")
            nc.sync.dma_start(t[:], bqk_d[p, i])
            bias_sb[(p, nm)] = t
    mask_sb = pconst.tile([128, 128], F32R, tag="mask")
    nc.sync.dma_start(mask_sb[:], mask_d)
    wout_sb = []
    for p in range(2):
        t = pconst.tile([128, E], F32R, tag=f"wout{p}", name=f"wout{p}")
        nc.sync.dma_start(t[:], wout_d[p])
        wout_sb.append(t)
    # xt[e][nq] = x[b].T e-chunk, n-chunk  [128, 512]
    xt = [[None] * NQ for _ in range(NE)]
    for nq in range(NQ):
        for e in range(NE):
            t = pconst.tile([128, 512], F32R, tag=f"xt{e}_{nq}",
                            name=f"xt{e}_{nq}")
            nc.sync.dma_start(t[:], xt_d[e][:, nq * 512:(nq + 1) * 512])
            xt[e][nq] = t

    # ---- emission helpers ----------------------------------------------
    qT = {}
    kT = {}
    for p in range(2):
        qT[p] = pqk.tile([128, N], F32R, tag=f"qT{p}", name=f"qT{p}")
        kT[p] = pqk.tile([128, N], F32R, tag=f"kT{p}", name=f"kT{p}")
    v_ext = [None] * NK
    saT = {}
    for p in range(2):
        saT[p] = psa.tile([128, N], F32R, tag=f"saT{p}", name=f"saT{p}")

    def emit_qkproj_group(p, which, nq):
        w_sb = wq_sb if which == "q" else wk_sb
        bias = bias_sb[(p, "bq" if which == "q" else "bk")]
        dst = qT[p] if which == "q" else kT[p]
        ps = pproj.tile([128, 512], F32, tag="proj", name="projps")
        for e in range(NE):
            nc.tensor.matmul(ps[:], w_sb[e][:, p * 128:(p + 1) * 128],
                             xt[e][nq][:], start=(e == 0), stop=(e == NE - 1))
        nc.vector.tensor_scalar_add(
            dst[:, nq * 512:(nq + 1) * 512], ps[:], bias[:])

    def emit_vproj_group(nk):
        ps = pproj.tile([128, 256], F32, tag="proj", name="vps")
        for e in range(NE):
            nc.tensor.matmul(
                ps[:], xt[e][nk // 4][:, (nk % 4) * 128:(nk % 4) * 128 + 128],
                wv_sb[e][:], start=(e == 0), stop=(e == NE - 1))
        vt = pvext.tile([128, 4 * 65], F32R, tag=f"vext{nk}", name=f"vext{nk}")
        nc.sync.dma_start(
            vt[:].rearrange("p (h d) -> p h d", h=4)[:, :, 64:65],
            ones_d[:].rearrange("p (h d) -> p h d", h=4))
        nc.vector.tensor_copy(
            vt[:].rearrange("p (h d) -> p h d", h=4)[:, :, 0:64],
            ps[:].rearrange("p (h d) -> p h d", h=4))
        v_ext[nk] = vt

    def attn_pass(p, hh, qlo, qhi, fillers):
        """One query pass [qlo,qhi) of one head; k-tiles 0..qhi//128-1.
        Software-pipelined: PV for k-tile kj is emitted after S/exp of
        kj+1 so the PE has queued work while ScalarE runs exp."""
        hloc = 2 * p + hh
        rb = hh * 64
        qis = [qlo // 512, qlo // 512 + 1]
        pv = {qi: ppv.tile([65, 512], F32, tag="pv", name=f"pv{qi}")
              for qi in qis}
        kjs = list(range(qhi // 128))
        pending = None

        def emit_pv(kj, esb):
            for qi in qis:
                if kj > 4 * qi + 3:
                    continue
                s = max(qi * 512, 128 * kj)
                nc.tensor.matmul(
                    pv[qi][:, s - qi * 512:512],
                    v_ext[kj][:, hloc * 65:hloc * 65 + 65],
                    esb[:, s - qlo:(qi + 1) * 512 - qlo],
                    start=(kj == 0), stop=(kj == 4 * qi + 3))
                if kj == 4 * qi + 3:
                    rcp = psmall.tile([1, 512], F32, tag="rcp", name="rcp")
                    nc.vector.reciprocal(rcp[:], pv[qi][64:65, :])
                    bc = psmall.tile([64, 512], F32, tag="bc", name="bc")
                    nc.gpsimd.partition_broadcast(bc[:], rcp[:])
                    nc.vector.tensor_mul(
                        saT[p][rb:rb + 64, qi * 512:(qi + 1) * 512],
                        pv[qi][0:64, :], bc[:])

        for step, kj in enumerate(kjs):
            q0 = 128 * kj
            lo = max(qlo, q0)
            w = qhi - lo
            esb = pesb.tile([128, qhi - qlo], F32R, tag="esb", name="esb")
            sps = pgen.tile([128, w], F32, tag="gen", name="sps")
            off = 0
            while off < w:
                n = min(512, w - off)
                nc.tensor.matmul(
                    sps[:, off:off + n],
                    kT[p][rb:rb + 64, q0:q0 + 128],
                    qT[p][rb:rb + 64, lo + off:lo + off + n],
                    start=True, stop=True)
                off += n
            nc.scalar.activation(esb[:, lo - qlo:qhi - qlo], sps[:], Exp)
            if q0 >= qlo:  # diagonal block lives in this pass
                nc.vector.tensor_mul(esb[:, q0 - qlo:q0 - qlo + 128],
                                     esb[:, q0 - qlo:q0 - qlo + 128],
                                     mask_sb[:])
            if step < len(fillers):
                fillers[step]()
            if pending is not None:
                emit_pv(*pending)
            pending = (kj, esb)
        emit_pv(*pending)

    # ---- schedule -------------------------------------------------------
    for nq in range(NQ):
        emit_qkproj_group(0, "k", nq)
        emit_qkproj_group(0, "q", nq)
    for nk in range(8):
        emit_vproj_group(nk)

    # head 0: fillers = v-proj nk 8..15 (pass A), qkproj p1 (pass B)
    attn_pass(0, 0, 0, 1024,
              [lambda nk=nk: emit_vproj_group(nk) for nk in range(8, 16)])
    p1_fill = [lambda nq=nq: emit_qkproj_group(1, "k", nq) for nq in range(NQ)]
    p1_fill += [lambda nq=nq: emit_qkproj_group(1, "q", nq) for nq in range(NQ)]
    attn_pass(0, 0, 1024, 2048, p1_fill)
    # remaining heads, no fillers
    attn_pass(0, 1, 0, 1024, [])
    attn_pass(0, 1, 1024, 2048, [])
    attn_pass(1, 0, 0, 1024, [])
    attn_pass(1, 0, 1024, 2048, [])
    attn_pass(1, 1, 0, 1024, [])
    attn_pass(1, 1, 1024, 2048, [])

    # ---- out projection (partial over this core's 256 features) ---------
    for nk in range(NK):
        ps = pgen.tile([128, 1024], F32, tag="gen", name="ops")
        for oc in range(2):
            for p in range(2):
                nc.tensor.matmul(ps[:, oc * 512:(oc + 1) * 512],
                                 saT[p][:, nk * 128:(nk + 1) * 128],
                                 wout_sb[p][:, oc * 512:(oc + 1) * 512],
                                 start=(p == 0), stop=(p == 1))
        ot = pout.tile([128, E], F32, tag="outsb", name="outsb")
        if nk % 2 == 0:
            nc.scalar.copy(ot[:], ps[:])
        else:
            nc.vector.tensor_copy(ot[:], ps[:])
        nc.sync.dma_start(out_d[nk], ot[:])


def build_nc(reps=1, loop=None):
    nc = bacc.Bacc("TRN2", target_bir_lowering=False, debug=False,
                   enable_asserts=True, num_devices=NCORES)
    xt_d = nc.dram_tensor("xt", [NE, 128, N], F32R, kind="ExternalInput").ap()
    wq_d = nc.dram_tensor("wq", [NE, 128, 256], F32R, kind="ExternalInput").ap()
    wk_d = nc.dram_tensor("wk", [NE, 128, 256], F32R, kind="ExternalInput").ap()
    wv_d = nc.dram_tensor("wv", [NE, 128, 256], F32R, kind="ExternalInput").ap()
    wout_d = nc.dram_tensor("wout", [2, 128, E], F32R, kind="ExternalInput").ap()
    bqk_d = nc.dram_tensor("bqk", [2, 2, 128, 1], F32, kind="ExternalInput").ap()
    mask_d = nc.dram_tensor("mask", [128, 128], F32R, kind="ExternalInput").ap()
    ones_d = nc.dram_tensor("ones", [128, 4], F32R, kind="ExternalInput").ap()
    out_d = nc.dram_tensor("out", [NK, 128, E], F32, kind="ExternalOutput").ap()
    dram = (xt_d, wq_d, wk_d, wv_d, wout_d, bqk_d, mask_d, ones_d, out_d)

    with tile.TileContext(nc) as tc:
        from contextlib import ExitStack
        with ExitStack() as ctx:
            pconst = ctx.enter_context(tc.tile_pool(name="const", bufs=1))
            pqk = ctx.enter_context(tc.tile_pool(name="qk", bufs=1))
            pvext = ctx.enter_context(tc.tile_pool(name="vext", bufs=1))
            psa = ctx.enter_context(tc.tile_pool(name="sa", bufs=1))
            pesb = ctx.enter_context(tc.tile_pool(name="esb", bufs=3))
            psmall = ctx.enter_context(tc.tile_pool(name="small", bufs=2))
            pout = ctx.enter_context(tc.tile_pool(name="outsb", bufs=2))
            ppv = ctx.enter_context(
                tc.tile_pool(name="pvps", bufs=2, space="PSUM"))
            pgen = ctx.enter_context(
                tc.tile_pool(name="gps", bufs=2, space="PSUM"))
            pproj = ctx.enter_context(
                tc.tile_pool(name="projps", bufs=2, space="PSUM"))
            pools = (pconst, pqk, pvext, psa, pesb, psmall, pout,
                     ppv, pgen, pproj)
            if loop is not None:
                with tc.For_i(0, loop, 1,
                              hint_engines=(mybir.EngineType.PE,
                                            mybir.EngineType.Activation,
                                            mybir.EngineType.DVE,
                                            mybir.EngineType.SP)):
                    _build_body(nc, tc, pools, dram, 0)
            else:
                for r in range(reps):
                    _build_body(nc, tc, pools, dram, r)
    nc.compile()
    return nc


def make_in_maps(x, Wqkv, bqkv, Wout):
    """Per-core input dicts. Shapes per reference: x[B,N,E], Wqkv[H,E,3HD],
    bqkv[H,3HD], Wout[E,E].  Split: cols 0:64=k, 64:128=q, 128:192=v."""
    Wk = Wqkv[:, :, 0:HD]
    Wq = Wqkv[:, :, HD:2 * HD] * (1.0 / np.sqrt(HD))
    Wv = Wqkv[:, :, 2 * HD:3 * HD]
    bk = bqkv[:, 0:HD]
    bq = bqkv[:, HD:2 * HD] * (1.0 / np.sqrt(HD))

    # expS^T tile rows are k, cols are q: keep k <= q -> upper triangular
    mask = np.triu(np.ones((128, 128), dtype=np.float32))
    in_maps = []
    for c in range(NCORES):
        b, hg = divmod(c, 4)
        hs = slice(4 * hg, 4 * hg + 4)

        xT = np.ascontiguousarray(x[b].T).reshape(NE, 128, N)

        def pack(w):  # [4,E,64] -> [NE,128,256]
            return np.ascontiguousarray(
                w.reshape(4, NE, 128, HD).transpose(1, 2, 0, 3)
                 .reshape(NE, 128, 256))

        wq = pack(Wq[hs])
        wk = pack(Wk[hs])
        wv = pack(Wv[hs])
        wout = np.ascontiguousarray(
            Wout[4 * hg * HD:(4 * hg + 4) * HD].reshape(2, 128, E))
        bqk = np.stack([
            np.stack([bq[4 * hg + 2 * p:4 * hg + 2 * p + 2].reshape(128),
                      bk[4 * hg + 2 * p:4 * hg + 2 * p + 2].reshape(128)])
            for p in range(2)]).reshape(2, 2, 128, 1)
        in_maps.append({
            "xt": xT.astype(np.float32),
            "wq": wq.astype(np.float32), "wk": wk.astype(np.float32),
            "wv": wv.astype(np.float32),
            "wout": wout.astype(np.float32),
            "bqk": bqk.astype(np.float32),
            "mask": mask,
            "ones": np.ones((128, 4), dtype=np.float32),
        })
    return in_maps


def combine(results, bqkv, Wout, bout):
    bv = bqkv[:, 2 * HD:3 * HD].reshape(E)          # concat over heads
    const_row = bv @ Wout + bout                     # [E]
    out = np.zeros((B, N, E), dtype=np.float32)
    for c in range(NCORES):
        b = c // 4
        out[b] += results[c]["out"].reshape(N, E)
    out += const_row[None, None, :].astype(np.float32)
    return out


def kernel(x, Wqkv, bqkv, Wout, bout):
    x = np.asarray(x, dtype=np.float32)
    Wqkv = np.asarray(Wqkv, dtype=np.float32)
    bqkv = np.asarray(bqkv, dtype=np.float32)
    Wout = np.asarray(Wout, dtype=np.float32)
    bout = np.asarray(bout, dtype=np.float32)

    if "nc" not in _CACHE:
        _CACHE["nc"] = build_nc(reps=1)
    nc = _CACHE["nc"]
    in_maps = make_in_maps(x, Wqkv, bqkv, Wout)
    res = bass_utils.run_bass_kernel_spmd(
        nc, in_maps, core_ids=list(range(NCORES)), trace=False)
    return combine(res.results, bqkv, Wout, bout)


# revision 41
# speedup vs baseline: 1.1706x; 1.1706x over previous
"""Causal self-attention (B=2, N=2048, E=1024, H=16, HD=64) on 8 trn2 NeuronCores.

Sharding: (batch, head-group) — core c handles batch c//4 and heads
4*(c%4) .. 4*(c%4)+3.  Each core computes its heads' QKV projections,
causal attention, and a partial out-projection over its 256 feature rows
of Wout; the host sums the 4 partials per batch and adds all biases that
are affine in the output (bout and the v-bias term, which is constant
because softmax rows sum to 1).

On-device layout avoids every transpose:
  - host feeds xT (E-major) so QK projections produce qT/kT [d, n] directly
  - S^T tiles [k, q] = kT-slice.T @ qT-slice (contraction over d)
  - exp on ScalarE (no max subtraction: logits are O(1) by construction)
  - PV uses expST tiles as lhsT with a ones-column appended to v to get the
    softmax denominator for free; normalization via gpsimd partition
    broadcast of the DVE reciprocal.
All matmuls run in float32r (full-rate fp32 mode, ~tf32 precision).

Perf notes (HW-measured):
  - K=64 matmuls are ~2.6x slower than K=128 at equal N, so the two heads
    of a pair are row-packed: their S^T matmuls go to PE row-groups (0,0)
    and (64,0) back-to-back and execute concurrently.
  - Attention works in (k-tile, q-chunk) steps with [128,<=512] score
    psums from a 4-slot pool — deep enough rotation that the PE isn't
    locked to ScalarE's exp cadence.
  - PSUM is the scarce resource (8 banks): one shared 4-slot [128,512]
    pool serves projections, scores and the out-projection; attention's
    four PV accumulators get their own scoped pool.
  - The first half of the out-projection is interleaved into the last two
    attention passes (its saT inputs are already final) to hide the
    output-DMA bandwidth.
"""

import numpy as np

import concourse.bass as bass
import concourse.tile as tile
from concourse import bacc, mybir
from concourse import bass_utils

B, N, E, H = 2, 2048, 1024, 16
HD = 64
NCORES = 8
NE = E // 128      # 8 e-chunks
NK = N // 128      # 16 k-tiles / n-chunks
NQ = N // 512      # 4 q-chunks of 512
F32 = mybir.dt.float32
F32R = mybir.dt.float32r

_CACHE = {}


def _build_body(nc, tc, pools, dram, rep, upto=4):
    xt_d, wq_d, wk_d, wv_d, wout_d, bqk_d, mask_d, ones_d, out_d = dram
    (pconst, pqk, pvext, psa, pesb, psmall, pout, psps) = pools
    Exp = mybir.ActivationFunctionType.Exp

    # ---- constant loads, ordered by first use: qk weights, first x
    # chunks, v weights, rest of x, wout last ------------------------------
    wq_sb, wk_sb, wv_sb = [], [], []
    for e in range(NE):
        for lst, src, nm in ((wk_sb, wk_d, "wk"), (wq_sb, wq_d, "wq")):
            t = pconst.tile([128, 256], F32R, tag=f"{nm}{e}", name=f"{nm}{e}")
            nc.sync.dma_start(t[:], src[e])
            lst.append(t)
    bias_sb = {}
    for p in range(2):
        for i, nm in enumerate(("bq", "bk")):
            t = pconst.tile([128, 1], F32, tag=f"{nm}{p}", name=f"{nm}{p}")
            nc.sync.dma_start(t[:], bqk_d[p, i])
            bias_sb[(p, nm)] = t
    xt = [[None] * NQ for _ in range(NE)]

    def load_xt(nq):
        for e in range(NE):
            t = pconst.tile([128, 512], F32R, tag=f"xt{e}_{nq}",
                            name=f"xt{e}_{nq}")
            nc.sync.dma_start(t[:], xt_d[e][:, nq * 512:(nq + 1) * 512])
            xt[e][nq] = t

    load_xt(0)
    load_xt(1)
    for e in range(NE):
        t = pconst.tile([128, 256], F32R, tag=f"wv{e}", name=f"wv{e}")
        nc.sync.dma_start(t[:], wv_d[e])
        wv_sb.append(t)
    load_xt(2)
    load_xt(3)
    mask_sb = pconst.tile([128, 128], F32R, tag="mask")
    nc.sync.dma_start(mask_sb[:], mask_d)
    wout_sb = []
    for p in range(2):
        t = pconst.tile([128, E], F32R, tag=f"wout{p}", name=f"wout{p}")
        nc.sync.dma_start(t[:], wout_d[p])
        wout_sb.append(t)

    qT = {}
    kT = {}
    for p in range(2):
        qT[p] = pqk.tile([128, N], F32R, tag=f"qT{p}", name=f"qT{p}")
        kT[p] = pqk.tile([128, N], F32R, tag=f"kT{p}", name=f"kT{p}")
    v_ext = [None] * NK
    saT = {}
    for p in range(2):
        saT[p] = psa.tile([128, N], F32R, tag=f"saT{p}", name=f"saT{p}")

    def emit_qkproj_group(p, which, nq):
        w_sb = wq_sb if which == "q" else wk_sb
        bias = bias_sb[(p, "bq" if which == "q" else "bk")]
        dst = qT[p] if which == "q" else kT[p]
        ps = psps.tile([128, 512], F32, tag="sps", name="projps")
        for e in range(NE):
            nc.tensor.matmul(ps[:], w_sb[e][:, p * 128:(p + 1) * 128],
                             xt[e][nq][:], start=(e == 0), stop=(e == NE - 1))
        nc.vector.tensor_scalar_add(
            dst[:, nq * 512:(nq + 1) * 512], ps[:], bias[:])

    def emit_vproj_group(nk):
        ps = psps.tile([128, 256], F32, tag="sps", name="vps")
        for e in range(NE):
            nc.tensor.matmul(
                ps[:], xt[e][nk // 4][:, (nk % 4) * 128:(nk % 4) * 128 + 128],
                wv_sb[e][:], start=(e == 0), stop=(e == NE - 1))
        vt = pvext.tile([128, 4 * 65], F32R, tag=f"vext{nk}", name=f"vext{nk}")
        nc.sync.dma_start(
            vt[:].rearrange("p (h d) -> p h d", h=4)[:, :, 64:65],
            ones_d[:].rearrange("p (h d) -> p h d", h=4))
        nc.vector.tensor_copy(
            vt[:].rearrange("p (h d) -> p h d", h=4)[:, :, 0:64],
            ps[:].rearrange("p (h d) -> p h d", h=4))
        v_ext[nk] = vt

    def emit_outproj_half(nk, oc):
        ps = psps.tile([128, 512], F32, tag="sps", name="ops")
        for p in range(2):
            nc.tensor.matmul(ps[:],
                             saT[p][:, nk * 128:(nk + 1) * 128],
                             wout_sb[p][:, oc * 512:(oc + 1) * 512],
                             start=(p == 0), stop=(p == 1))
        ot = pout.tile([128, 512], F32, tag="outsb", name="outsb")
        nc.vector.tensor_copy(ot[:], ps[:])
        nc.sync.dma_start(out_d[nk][:, oc * 512:(oc + 1) * 512], ot[:])

    def attn_qi_pass(qi, ppv, fillers, do_exp=True, do_pv=True,
                     do_norm=True):
        """Attention for query chunk qi, BOTH pairs, all four heads.
        Per k-tile step both pairs' score matmuls are emitted (each pair's
        two heads row-packed into PE row-groups (0,0)/(64,0)); one exp per
        pair per step; PV lags one step.  While ScalarE exponentiates one
        pair's tile the PE streams the other pair's work."""
        pv = {(p, hh): ppv.tile([65, 512], F32, tag="pv",
                                name=f"pv{p}_{hh}")
              for p in range(2) for hh in range(2)}
        pending = []
        nfill = 0

        def emit_pv(kj, lo, hi, esbs):
            for p in range(2):
                for hh in range(2):
                    hloc = 2 * p + hh
                    nc.tensor.matmul(
                        pv[(p, hh)][:, lo - qi * 512:512],
                        v_ext[kj][:, hloc * 65:hloc * 65 + 65],
                        esbs[p][:, hh * 512:hh * 512 + hi - lo],
                        start=(kj == 0), stop=(kj == 4 * qi + 3))
                    if kj == 4 * qi + 3 and do_norm:
                        rcp = psmall.tile([1, 512], F32, tag="rcp",
                                          name="rcp")
                        nc.vector.reciprocal(rcp[:], pv[(p, hh)][64:65, :])
                        bc = psmall.tile([64, 512], F32, tag="bc", name="bc")
                        nc.gpsimd.partition_broadcast(bc[:], rcp[:])
                        nc.vector.tensor_mul(
                            saT[p][hh * 64:hh * 64 + 64,
                                   qi * 512:(qi + 1) * 512],
                            pv[(p, hh)][0:64, :], bc[:])

        hi = qi * 512 + 512
        for kj in range(4 * qi + 4):
            q0 = 128 * kj
            lo = max(qi * 512, q0)
            w = hi - lo
            esbs = []
            for p in range(2):
                # h0 scores at psum cols [0,w), h1 at [512, 512+w): separate
                # banks so the two row-group matmuls run concurrently, and
                # ONE exp covers both heads.
                esb = pesb.tile([128, 1024], F32R, tag="esb", name="esb")
                sps = psps.tile([128, 512 + w], F32, tag="sps", name="sps")
                for hh in range(2):
                    rb = hh * 64
                    nc.tensor.matmul(
                        sps[:, hh * 512:hh * 512 + w],
                        kT[p][rb:rb + 64, q0:q0 + 128],
                        qT[p][rb:rb + 64, lo:hi], start=True, stop=True)
                if do_exp:
                    nc.scalar.activation(esb[:, 0:512 + w], sps[:], Exp)
                elif do_exp is not None:
                    nc.vector.tensor_copy(esb[:, 0:512 + w], sps[:])
                if lo == q0 and do_exp is not None:  # diagonal block
                    for hh in range(2):
                        nc.vector.tensor_mul(
                            esb[:, hh * 512:hh * 512 + 128],
                            esb[:, hh * 512:hh * 512 + 128], mask_sb[:])
                esbs.append(esb)
            if kj % 2 == 1 and nfill < len(fillers):
                fillers[nfill]()
                nfill += 1
            pending.append((kj, lo, hi, esbs))
            if do_pv and len(pending) > 1:
                emit_pv(*pending.pop(0))
        if do_pv:
            while pending:
                emit_pv(*pending.pop(0))
        elif do_exp is not None:
            for p in range(2):
                for hh in range(2):
                    nc.vector.tensor_copy(
                        saT[p][hh * 64:hh * 64 + 64, qi * 512:qi * 512 + 512],
                        pending[-1][3][p][:, hh * 512:hh * 512 + 512][0:64, :])
        else:
            for p in range(2):
                for hh in range(2):
                    nc.vector.tensor_copy(
                        saT[p][hh * 64:hh * 64 + 64, qi * 512:qi * 512 + 512],
                        qT[p][hh * 64:hh * 64 + 64, 0:512])
        while nfill < len(fillers):
            fillers[nfill]()
            nfill += 1

    # ---- schedule -------------------------------------------------------
    if upto <= 1:   # loads only: consume tiles so the DMAs stay live
        dummy = pout.tile([128, 512], F32, tag="outsb", name="outsb")
        nc.vector.tensor_copy(dummy[:], xt[0][0][:])
        nc.sync.dma_start(out_d[0][:, 0:512], dummy[:])
        return

    for nq in range(NQ):
        emit_qkproj_group(0, "k", nq)
        emit_qkproj_group(0, "q", nq)
        emit_qkproj_group(1, "k", nq)
        emit_qkproj_group(1, "q", nq)
    for nk in range(NK):
        emit_vproj_group(nk)

    if upto == 2:
        dummy = pout.tile([128, 512], F32, tag="outsb", name="outsb")
        nc.vector.tensor_copy(dummy[:], qT[1][:, 0:512])
        nc.sync.dma_start(out_d[0][:, 0:512], dummy[:])
        return

    kw = {}
    if upto in (5, 6, 7):
        kw = dict(do_exp=(True if upto >= 6 else None),
                  do_pv=(upto >= 7), do_norm=(upto >= 7))

    do_out = (upto == 4)
    with tc.tile_pool(name="pvps", bufs=4, space="PSUM") as ppv:
        for qi in range(NQ):
            fillers = []
            if do_out and qi >= 1:
                # saT[:, (qi-1) block] is final: overlap its out-projection
                # with this qi pass.
                fillers = [lambda nk=nk, oc=oc: emit_outproj_half(nk, oc)
                           for nk in range(4 * (qi - 1), 4 * qi)
                           for oc in range(2)]
            attn_qi_pass(qi, ppv, fillers, **kw)

    if not do_out:   # skip out-projection; flush saT
        dummy = pout.tile([128, 512], F32, tag="outsb", name="outsb")
        nc.vector.tensor_copy(dummy[:], saT[0][:, 0:512])
        nc.sync.dma_start(out_d[0][:, 0:512], dummy[:])
        dummy2 = pout.tile([128, 512], F32, tag="outsb", name="outsb2")
        nc.vector.tensor_copy(dummy2[:], saT[1][:, 0:512])
        nc.sync.dma_start(out_d[1][:, 0:512], dummy2[:])
        return

    for nk in range(12, NK):
        for oc in range(2):
            emit_outproj_half(nk, oc)


def build_nc(reps=1, loop=None, upto=4):
    nc = bacc.Bacc("TRN2", target_bir_lowering=False, debug=False,
                   enable_asserts=True, num_devices=NCORES)
    xt_d = nc.dram_tensor("xt", [NE, 128, N], F32R, kind="ExternalInput").ap()
    wq_d = nc.dram_tensor("wq", [NE, 128, 256], F32R, kind="ExternalInput").ap()
    wk_d = nc.dram_tensor("wk", [NE, 128, 256], F32R, kind="ExternalInput").ap()
    wv_d = nc.dram_tensor("wv", [NE, 128, 256], F32R, kind="ExternalInput").ap()
    wout_d = nc.dram_tensor("wout", [2, 128, E], F32R, kind="ExternalInput").ap()
    bqk_d = nc.dram_tensor("bqk", [2, 2, 128, 1], F32, kind="ExternalInput").ap()
    mask_d = nc.dram_tensor("mask", [128, 128], F32R, kind="ExternalInput").ap()
    ones_d = nc.dram_tensor("ones", [128, 4], F32R, kind="ExternalInput").ap()
    out_d = nc.dram_tensor("out", [NK, 128, E], F32, kind="ExternalOutput").ap()
    dram = (xt_d, wq_d, wk_d, wv_d, wout_d, bqk_d, mask_d, ones_d, out_d)

    with tile.TileContext(nc) as tc:
        from contextlib import ExitStack
        with ExitStack() as ctx:
            pconst = ctx.enter_context(tc.tile_pool(name="const", bufs=1))
            pqk = ctx.enter_context(tc.tile_pool(name="qk", bufs=1))
            pvext = ctx.enter_context(tc.tile_pool(name="vext", bufs=1))
            psa = ctx.enter_context(tc.tile_pool(name="sa", bufs=1))
            pesb = ctx.enter_context(tc.tile_pool(name="esb", bufs=7))
            psmall = ctx.enter_context(tc.tile_pool(name="small", bufs=3))
            pout = ctx.enter_context(tc.tile_pool(name="outsb", bufs=3))
            psps = ctx.enter_context(
                tc.tile_pool(name="sps", bufs=2, space="PSUM"))
            pools = (pconst, pqk, pvext, psa, pesb, psmall, pout, psps)
            if loop is not None:
                with tc.For_i(0, loop, 1,
                              hint_engines=(mybir.EngineType.PE,
                                            mybir.EngineType.Activation,
                                            mybir.EngineType.DVE,
                                            mybir.EngineType.SP)):
                    _build_body(nc, tc, pools, dram, 0, upto=upto)
            else:
                for r in range(reps):
                    _build_body(nc, tc, pools, dram, r, upto=upto)
    nc.compile()
    return nc


def make_in_maps(x, Wqkv, bqkv, Wout):
    """Per-core input dicts. Shapes per reference: x[B,N,E], Wqkv[H,E,3HD],
    bqkv[H,3HD], Wout[E,E].  Split: cols 0:64=k, 64:128=q, 128:192=v."""
    Wk = Wqkv[:, :, 0:HD]
    Wq = Wqkv[:, :, HD:2 * HD] * (1.0 / np.sqrt(HD))
    Wv = Wqkv[:, :, 2 * HD:3 * HD]
    bk = bqkv[:, 0:HD]
    bq = bqkv[:, HD:2 * HD] * (1.0 / np.sqrt(HD))

    # expS^T tile rows are k, cols are q: keep k <= q -> upper triangular
    mask = np.triu(np.ones((128, 128), dtype=np.float32))
    in_maps = []
    for c in range(NCORES):
        b, hg = divmod(c, 4)
        hs = slice(4 * hg, 4 * hg + 4)

        xT = np.ascontiguousarray(x[b].T).reshape(NE, 128, N)

        def pack(w):  # [4,E,64] -> [NE,128,256]
            return np.ascontiguousarray(
                w.reshape(4, NE, 128, HD).transpose(1, 2, 0, 3)
                 .reshape(NE, 128, 256))

        wq = pack(Wq[hs])
        wk = pack(Wk[hs])
        wv = pack(Wv[hs])
        wout = np.ascontiguousarray(
            Wout[4 * hg * HD:(4 * hg + 4) * HD].reshape(2, 128, E))
        bqk = np.stack([
            np.stack([bq[4 * hg + 2 * p:4 * hg + 2 * p + 2].reshape(128),
                      bk[4 * hg + 2 * p:4 * hg + 2 * p + 2].reshape(128)])
            for p in range(2)]).reshape(2, 2, 128, 1)
        in_maps.append({
            "xt": xT.astype(np.float32),
            "wq": wq.astype(np.float32), "wk": wk.astype(np.float32),
            "wv": wv.astype(np.float32),
            "wout": wout.astype(np.float32),
            "bqk": bqk.astype(np.float32),
            "mask": mask,
            "ones": np.ones((128, 4), dtype=np.float32),
        })
    return in_maps


def combine(results, bqkv, Wout, bout):
    bv = bqkv[:, 2 * HD:3 * HD].reshape(E)          # concat over heads
    const_row = bv @ Wout + bout                     # [E]
    out = np.zeros((B, N, E), dtype=np.float32)
    for c in range(NCORES):
        b = c // 4
        out[b] += results[c]["out"].reshape(N, E)
    out += const_row[None, None, :].astype(np.float32)
    return out


def kernel(x, Wqkv, bqkv, Wout, bout):
    x = np.asarray(x, dtype=np.float32)
    Wqkv = np.asarray(Wqkv, dtype=np.float32)
    bqkv = np.asarray(bqkv, dtype=np.float32)
    Wout = np.asarray(Wout, dtype=np.float32)
    bout = np.asarray(bout, dtype=np.float32)

    if "nc" not in _CACHE:
        _CACHE["nc"] = build_nc(reps=1)
    nc = _CACHE["nc"]
    in_maps = make_in_maps(x, Wqkv, bqkv, Wout)
    res = bass_utils.run_bass_kernel_spmd(
        nc, in_maps, core_ids=list(range(NCORES)), trace=False)
    return combine(res.results, bqkv, Wout, bout)


# revision 47
# speedup vs baseline: 1.2003x; 1.0253x over previous
"""Causal self-attention (B=2, N=2048, E=1024, H=16, HD=64) on 8 trn2 NeuronCores.

Sharding: (batch, head-group) — core c handles batch c//4 and heads
4*(c%4) .. 4*(c%4)+3.  Each core computes its heads' QKV projections,
causal attention, and a partial out-projection over its 256 feature rows
of Wout; the host sums the 4 partials per batch and adds all biases that
are affine in the output (bout and the v-bias term, which is constant
because softmax rows sum to 1).

On-device layout avoids every transpose:
  - host feeds xT (E-major) so QK projections produce qT/kT [d, n] directly
  - S^T tiles [k, q] = kT-slice.T @ qT-slice (contraction over d)
  - exp on ScalarE (no max subtraction: logits are O(1) by construction)
  - PV uses expST tiles as lhsT with a ones-column appended to v to get the
    softmax denominator for free; normalization via gpsimd partition
    broadcast of the DVE reciprocal.
All matmuls run in float32r (full-rate fp32 mode, ~tf32 precision).

Perf notes (HW-measured):
  - K=64 matmuls are ~2.6x slower than K=128 at equal N, so the two heads
    of a pair are row-packed: their S^T matmuls go to PE row-groups (0,0)
    and (64,0) back-to-back and execute concurrently.
  - Attention works in (k-tile, q-chunk) steps with [128,<=512] score
    psums from a 4-slot pool — deep enough rotation that the PE isn't
    locked to ScalarE's exp cadence.
  - PSUM is the scarce resource (8 banks): one shared 4-slot [128,512]
    pool serves projections, scores and the out-projection; attention's
    four PV accumulators get their own scoped pool.
  - The first half of the out-projection is interleaved into the last two
    attention passes (its saT inputs are already final) to hide the
    output-DMA bandwidth.
"""

import numpy as np

import concourse.bass as bass
import concourse.tile as tile
from concourse import bacc, mybir
from concourse import bass_utils

B, N, E, H = 2, 2048, 1024, 16
HD = 64
NCORES = 8
NE = E // 128      # 8 e-chunks
NK = N // 128      # 16 k-tiles / n-chunks
NQ = N // 512      # 4 q-chunks of 512
F32 = mybir.dt.float32
F32R = mybir.dt.float32r

_CACHE = {}


def _build_body(nc, tc, pools, dram, rep, upto=4):
    xt_d, wq_d, wk_d, wv_d, wout_d, bqk_d, mask_d, ones_d, out_d = dram
    (pconst, pqk, pvext, psa, pesb, psmall, pout, psps) = pools
    Exp = mybir.ActivationFunctionType.Exp

    # ---- constant loads, ordered by first use: qk weights, first x
    # chunks, v weights, rest of x, wout last ------------------------------
    wq_sb, wk_sb, wv_sb = [], [], []
    for e in range(NE):
        for lst, src, nm in ((wk_sb, wk_d, "wk"), (wq_sb, wq_d, "wq")):
            t = pconst.tile([128, 256], F32R, tag=f"{nm}{e}", name=f"{nm}{e}")
            nc.sync.dma_start(t[:], src[e])
            lst.append(t)
    bias_sb = {}
    for p in range(2):
        for i, nm in enumerate(("bq", "bk")):
            t = pconst.tile([128, 1], F32, tag=f"{nm}{p}", name=f"{nm}{p}")
            nc.sync.dma_start(t[:], bqk_d[p, i])
            bias_sb[(p, nm)] = t
    xt = [[None] * NQ for _ in range(NE)]

    def load_xt(nq):
        for e in range(NE):
            t = pconst.tile([128, 512], F32R, tag=f"xt{e}_{nq}",
                            name=f"xt{e}_{nq}")
            nc.sync.dma_start(t[:], xt_d[e][:, nq * 512:(nq + 1) * 512])
            xt[e][nq] = t

    load_xt(0)
    load_xt(1)
    for e in range(NE):
        t = pconst.tile([128, 256], F32R, tag=f"wv{e}", name=f"wv{e}")
        nc.sync.dma_start(t[:], wv_d[e])
        wv_sb.append(t)
    load_xt(2)
    load_xt(3)
    mask_sb = pconst.tile([128, 128], F32R, tag="mask")
    nc.sync.dma_start(mask_sb[:], mask_d)
    wout_sb = []
    for p in range(2):
        t = pconst.tile([128, E], F32R, tag=f"wout{p}", name=f"wout{p}")
        nc.sync.dma_start(t[:], wout_d[p])
        wout_sb.append(t)

    qT = {}
    kT = {}
    for p in range(2):
        qT[p] = pqk.tile([128, N], F32R, tag=f"qT{p}", name=f"qT{p}")
        kT[p] = pqk.tile([128, N], F32R, tag=f"kT{p}", name=f"kT{p}")
    v_ext = [None] * NK
    saT = {}
    for p in range(2):
        saT[p] = psa.tile([128, N], F32R, tag=f"saT{p}", name=f"saT{p}")

    def emit_qkproj_group(p, which, nq):
        w_sb = wq_sb if which == "q" else wk_sb
        bias = bias_sb[(p, "bq" if which == "q" else "bk")]
        dst = qT[p] if which == "q" else kT[p]
        ps = psps.tile([128, 512], F32, tag="sps", name="projps")
        for e in range(NE):
            nc.tensor.matmul(ps[:], w_sb[e][:, p * 128:(p + 1) * 128],
                             xt[e][nq][:], start=(e == 0), stop=(e == NE - 1))
        nc.vector.tensor_scalar_add(
            dst[:, nq * 512:(nq + 1) * 512], ps[:], bias[:])

    def emit_vproj_group(nk):
        ps = psps.tile([128, 256], F32, tag="sps", name="vps")
        for e in range(NE):
            nc.tensor.matmul(
                ps[:], xt[e][nk // 4][:, (nk % 4) * 128:(nk % 4) * 128 + 128],
                wv_sb[e][:], start=(e == 0), stop=(e == NE - 1))
        vt = pvext.tile([128, 4 * 65], F32R, tag=f"vext{nk}", name=f"vext{nk}")
        nc.sync.dma_start(
            vt[:].rearrange("p (h d) -> p h d", h=4)[:, :, 64:65],
            ones_d[:].rearrange("p (h d) -> p h d", h=4))
        nc.vector.tensor_copy(
            vt[:].rearrange("p (h d) -> p h d", h=4)[:, :, 0:64],
            ps[:].rearrange("p (h d) -> p h d", h=4))
        v_ext[nk] = vt

    def emit_outproj_half(nk, oc):
        ps = psps.tile([128, 512], F32, tag="sps", name="ops")
        for p in range(2):
            nc.tensor.matmul(ps[:],
                             saT[p][:, nk * 128:(nk + 1) * 128],
                             wout_sb[p][:, oc * 512:(oc + 1) * 512],
                             start=(p == 0), stop=(p == 1))
        ot = pout.tile([128, 512], F32, tag="outsb", name="outsb")
        nc.vector.tensor_copy(ot[:], ps[:])
        nc.sync.dma_start(out_d[nk][:, oc * 512:(oc + 1) * 512], ot[:])

    def attn_qi_pass(qi, ppv, fillers, do_exp=True, do_pv=True,
                     do_norm=True):
        """Attention for query chunk qi, BOTH pairs, all four heads.
        Per k-tile step both pairs' score matmuls are emitted (each pair's
        two heads row-packed into PE row-groups (0,0)/(64,0)); one exp per
        pair per step; PV lags one step.  While ScalarE exponentiates one
        pair's tile the PE streams the other pair's work."""
        pv = {(p, hh): ppv.tile([65, 512], F32, tag="pv",
                                name=f"pv{p}_{hh}")
              for p in range(2) for hh in range(2)}
        pending = []
        nfill = 0

        def emit_pv(kj, lo, hi, esbs):
            for p in range(2):
                for hh in range(2):
                    hloc = 2 * p + hh
                    nc.tensor.matmul(
                        pv[(p, hh)][:, lo - qi * 512:512],
                        v_ext[kj][:, hloc * 65:hloc * 65 + 65],
                        esbs[p][:, hh * 512:hh * 512 + hi - lo],
                        start=(kj == 0), stop=(kj == 4 * qi + 3))
                    if kj == 4 * qi + 3 and do_norm:
                        rcp = psmall.tile([1, 512], F32, tag="rcp",
                                          name="rcp")
                        nc.vector.reciprocal(rcp[:], pv[(p, hh)][64:65, :])
                        bc = psmall.tile([64, 512], F32, tag="bc", name="bc")
                        nc.gpsimd.partition_broadcast(bc[:], rcp[:])
                        nc.vector.tensor_mul(
                            saT[p][hh * 64:hh * 64 + 64,
                                   qi * 512:(qi + 1) * 512],
                            pv[(p, hh)][0:64, :], bc[:])

        hi = qi * 512 + 512
        for kj in range(4 * qi + 4):
            q0 = 128 * kj
            lo = max(qi * 512, q0)
            w = hi - lo
            esbs = []
            for p in range(2):
                # h0 scores at psum cols [0,w), h1 at [512, 512+w): separate
                # banks so the two row-group matmuls run concurrently, and
                # ONE exp covers both heads.
                esb = pesb.tile([128, 1024], F32R, tag="esb", name="esb")
                sps = psps.tile([128, 512 + w], F32, tag="sps", name="sps")
                for hh in range(2):
                    rb = hh * 64
                    nc.tensor.matmul(
                        sps[:, hh * 512:hh * 512 + w],
                        kT[p][rb:rb + 64, q0:q0 + 128],
                        qT[p][rb:rb + 64, lo:hi], start=True, stop=True)
                if do_exp:
                    nc.scalar.activation(esb[:, 0:512 + w], sps[:], Exp)
                elif do_exp is not None:
                    nc.vector.tensor_copy(esb[:, 0:512 + w], sps[:])
                if lo == q0 and do_exp is not None:  # diagonal block
                    for hh in range(2):
                        nc.vector.tensor_mul(
                            esb[:, hh * 512:hh * 512 + 128],
                            esb[:, hh * 512:hh * 512 + 128], mask_sb[:])
                esbs.append(esb)
            if nfill < len(fillers):
                fillers[nfill]()
                nfill += 1
            pending.append((kj, lo, hi, esbs))
            if do_pv and len(pending) > 1:
                emit_pv(*pending.pop(0))
        if do_pv:
            while pending:
                emit_pv(*pending.pop(0))
        elif do_exp is not None:
            for p in range(2):
                for hh in range(2):
                    nc.vector.tensor_copy(
                        saT[p][hh * 64:hh * 64 + 64, qi * 512:qi * 512 + 512],
                        pending[-1][3][p][:, hh * 512:hh * 512 + 512][0:64, :])
        else:
            for p in range(2):
                for hh in range(2):
                    nc.vector.tensor_copy(
                        saT[p][hh * 64:hh * 64 + 64, qi * 512:qi * 512 + 512],
                        qT[p][hh * 64:hh * 64 + 64, 0:512])
        while nfill < len(fillers):
            fillers[nfill]()
            nfill += 1

    # ---- schedule -------------------------------------------------------
    if upto <= 1:   # loads only: consume tiles so the DMAs stay live
        dummy = pout.tile([128, 512], F32, tag="outsb", name="outsb")
        nc.vector.tensor_copy(dummy[:], xt[0][0][:])
        nc.sync.dma_start(out_d[0][:, 0:512], dummy[:])
        return

    # Minimal projection prefix: attention pass qi only reads qT/kT chunk
    # columns <= 512*(qi+1) and v_ext[<=4qi+3], so only the nq=0 chunk and
    # v_ext[0..3] must exist before qi=0 starts; the rest streams in as
    # PE filler inside earlier passes.
    # Interleaving projection groups into the attention passes was tried
    # and measured WORSE (268-273us vs 246us): each 8-matmul projection
    # group competes for the two shared score-psum slots and stalls the
    # attention pipeline. Keep the projections as a serial prefix (they
    # overlap the input DMA, which is the real floor there).
    for nq in range(NQ):
        for p in (0, 1):
            emit_qkproj_group(p, "k", nq)
            emit_qkproj_group(p, "q", nq)
    for nk in range(NK):
        emit_vproj_group(nk)
    if upto == 2:
        dummy = pout.tile([128, 512], F32, tag="outsb", name="outsb")
        nc.vector.tensor_copy(dummy[:], qT[1][:, 0:512])
        nc.sync.dma_start(out_d[0][:, 0:512], dummy[:])
        return

    kw = {}
    if upto in (5, 6, 7):
        kw = dict(do_exp=(True if upto >= 6 else None),
                  do_pv=(upto >= 7), do_norm=(upto >= 7))

    do_out = (upto == 4)
    with tc.tile_pool(name="pvps", bufs=4, space="PSUM") as ppv:
        for qi in range(NQ):
            fillers = []
            if do_out and qi >= 1:
                # saT[:, (qi-1) block] is final: overlap its out-projection
                # with this qi pass.
                fillers += [lambda nk=nk, oc=oc: emit_outproj_half(nk, oc)
                            for nk in range(4 * (qi - 1), 4 * qi)
                            for oc in range(2)]
            attn_qi_pass(qi, ppv, fillers, **kw)

    if not do_out:   # skip out-projection; flush saT
        dummy = pout.tile([128, 512], F32, tag="outsb", name="outsb")
        nc.vector.tensor_copy(dummy[:], saT[0][:, 0:512])
        nc.sync.dma_start(out_d[0][:, 0:512], dummy[:])
        dummy2 = pout.tile([128, 512], F32, tag="outsb", name="outsb2")
        nc.vector.tensor_copy(dummy2[:], saT[1][:, 0:512])
        nc.sync.dma_start(out_d[1][:, 0:512], dummy2[:])
        return

    for nk in range(12, NK):
        for oc in range(2):
            emit_outproj_half(nk, oc)


def build_nc(reps=1, loop=None, upto=4):
    nc = bacc.Bacc("TRN2", target_bir_lowering=False, debug=False,
                   enable_asserts=True, num_devices=NCORES)
    xt_d = nc.dram_tensor("xt", [NE, 128, N], F32R, kind="ExternalInput").ap()
    wq_d = nc.dram_tensor("wq", [NE, 128, 256], F32R, kind="ExternalInput").ap()
    wk_d = nc.dram_tensor("wk", [NE, 128, 256], F32R, kind="ExternalInput").ap()
    wv_d = nc.dram_tensor("wv", [NE, 128, 256], F32R, kind="ExternalInput").ap()
    wout_d = nc.dram_tensor("wout", [2, 128, E], F32R, kind="ExternalInput").ap()
    bqk_d = nc.dram_tensor("bqk", [2, 2, 128, 1], F32, kind="ExternalInput").ap()
    mask_d = nc.dram_tensor("mask", [128, 128], F32R, kind="ExternalInput").ap()
    ones_d = nc.dram_tensor("ones", [128, 4], F32R, kind="ExternalInput").ap()
    out_d = nc.dram_tensor("out", [NK, 128, E], F32, kind="ExternalOutput").ap()
    dram = (xt_d, wq_d, wk_d, wv_d, wout_d, bqk_d, mask_d, ones_d, out_d)

    with tile.TileContext(nc) as tc:
        from contextlib import ExitStack
        with ExitStack() as ctx:
            pconst = ctx.enter_context(tc.tile_pool(name="const", bufs=1))
            pqk = ctx.enter_context(tc.tile_pool(name="qk", bufs=1))
            pvext = ctx.enter_context(tc.tile_pool(name="vext", bufs=1))
            psa = ctx.enter_context(tc.tile_pool(name="sa", bufs=1))
            pesb = ctx.enter_context(tc.tile_pool(name="esb", bufs=7))
            psmall = ctx.enter_context(tc.tile_pool(name="small", bufs=3))
            pout = ctx.enter_context(tc.tile_pool(name="outsb", bufs=3))
            psps = ctx.enter_context(
                tc.tile_pool(name="sps", bufs=2, space="PSUM"))
            pools = (pconst, pqk, pvext, psa, pesb, psmall, pout, psps)
            if loop is not None:
                with tc.For_i(0, loop, 1,
                              hint_engines=(mybir.EngineType.PE,
                                            mybir.EngineType.Activation,
                                            mybir.EngineType.DVE,
                                            mybir.EngineType.SP)):
                    _build_body(nc, tc, pools, dram, 0, upto=upto)
            else:
                for r in range(reps):
                    _build_body(nc, tc, pools, dram, r, upto=upto)
    nc.compile()
    return nc


def make_in_maps(x, Wqkv, bqkv, Wout):
    """Per-core input dicts. Shapes per reference: x[B,N,E], Wqkv[H,E,3HD],
    bqkv[H,3HD], Wout[E,E].  Split: cols 0:64=k, 64:128=q, 128:192=v."""
    Wk = Wqkv[:, :, 0:HD]
    Wq = Wqkv[:, :, HD:2 * HD] * (1.0 / np.sqrt(HD))
    Wv = Wqkv[:, :, 2 * HD:3 * HD]
    bk = bqkv[:, 0:HD]
    bq = bqkv[:, HD:2 * HD] * (1.0 / np.sqrt(HD))

    # expS^T tile rows are k, cols are q: keep k <= q -> upper triangular
    mask = np.triu(np.ones((128, 128), dtype=np.float32))
    in_maps = []
    for c in range(NCORES):
        b, hg = divmod(c, 4)
        hs = slice(4 * hg, 4 * hg + 4)

        xT = np.ascontiguousarray(x[b].T).reshape(NE, 128, N)

        def pack(w):  # [4,E,64] -> [NE,128,256]
            return np.ascontiguousarray(
                w.reshape(4, NE, 128, HD).transpose(1, 2, 0, 3)
                 .reshape(NE, 128, 256))

        wq = pack(Wq[hs])
        wk = pack(Wk[hs])
        wv = pack(Wv[hs])
        wout = np.ascontiguousarray(
            Wout[4 * hg * HD:(4 * hg + 4) * HD].reshape(2, 128, E))
        bqk = np.stack([
            np.stack([bq[4 * hg + 2 * p:4 * hg + 2 * p + 2].reshape(128),
                      bk[4 * hg + 2 * p:4 * hg + 2 * p + 2].reshape(128)])
            for p in range(2)]).reshape(2, 2, 128, 1)
        in_maps.append({
            "xt": xT.astype(np.float32),
            "wq": wq.astype(np.float32), "wk": wk.astype(np.float32),
            "wv": wv.astype(np.float32),
            "wout": wout.astype(np.float32),
            "bqk": bqk.astype(np.float32),
            "mask": mask,
            "ones": np.ones((128, 4), dtype=np.float32),
        })
    return in_maps


def combine(results, bqkv, Wout, bout):
    bv = bqkv[:, 2 * HD:3 * HD].reshape(E)          # concat over heads
    const_row = bv @ Wout + bout                     # [E]
    out = np.zeros((B, N, E), dtype=np.float32)
    for c in range(NCORES):
        b = c // 4
        out[b] += results[c]["out"].reshape(N, E)
    out += const_row[None, None, :].astype(np.float32)
    return out


def kernel(x, Wqkv, bqkv, Wout, bout):
    x = np.asarray(x, dtype=np.float32)
    Wqkv = np.asarray(Wqkv, dtype=np.float32)
    bqkv = np.asarray(bqkv, dtype=np.float32)
    Wout = np.asarray(Wout, dtype=np.float32)
    bout = np.asarray(bout, dtype=np.float32)

    if "nc" not in _CACHE:
        _CACHE["nc"] = build_nc(reps=1)
    nc = _CACHE["nc"]
    in_maps = make_in_maps(x, Wqkv, bqkv, Wout)
    res = bass_utils.run_bass_kernel_spmd(
        nc, in_maps, core_ids=list(range(NCORES)), trace=False)
    return combine(res.results, bqkv, Wout, bout)
